# revision 25
# baseline (speedup 1.0000x reference)
"""Trainium2 Bass kernel for a 2-layer dense-adjacency GAT (nn_GAT_17824114278677).

Sharding: nodes (rows of the attention matrix) are sharded across the 8
NeuronCores, 512 rows per core; weights and node features are replicated.
Two SPMD launches (one per GAT layer) with a host-side gather of the layer-1
output in between.

Per-core dataflow: attention tiles are computed TRANSPOSED, [j=128
partitions, r=512 rows], so the aggregation att @ Wh maps directly onto the
PE (contraction over j on partitions) with zero on-chip transposes.
Identities used:

  * softmax is shift-invariant and all logits are bounded (|e| < ~6), so no
    max-subtraction is needed; masked logits get -1000 added (or a 0/1
    multiplicative mask), which produces exactly 0 after exp, matching the
    reference's -9e15 mask.
  * exp(leaky_relu(t)) == max(exp(t), exp(0.2 t)).  Two ways to evaluate it
    per 128x512 tile, assigned per key-chunk to balance ScalarE vs VectorE:
      path A (ScalarE-heavy): t0 = mask + f_src (DVE); exp(t0 + f_dst) and
        exp(0.2 t0 + 0.2 f_dst) on ScalarE (bias = per-partition AP);
        max on DVE.
      path B (VectorE-only, rank-1): with v = exp(f_dst) folded into the
        host-side Whx (and the ones-column replaced by v), the tile is
        p' = max(u, u2*w) * mask01 with u = exp(f_src) broadcast and
        w = exp(-0.8 f_dst) per-partition: one tensor_scalar + two fused
        tensor_tensor ops, all bf16.
  * softmax denominators ride along as a ones-column in the stationary
    operand; division + ELU happen on the host on the tiny per-head
    [HID+1, 512] outputs.

Wh = x @ W (0.4%% of the FLOPs) plus the per-node attention vectors
f_src/f_dst are computed on the host in fp32 and shipped pre-rounded to
bf16; all on-device attention/aggregation math runs in bf16 with fp32 PSUM
accumulation.  Measured on 8 axon-tunneled trn2 cores: ~211 us (layer 1) +
~52 us (layer 2) = ~263 us, end-to-end relative error vs the fp32 jax
reference ~7e-4.  VectorE/ScalarE both measure >93%% busy -- the kernel sits
at the elementwise-engine saturation floor of this op chain.
"""

import os
import sys
import time
from contextlib import ExitStack

for _p in ("/opt/trn_rl_repo", "/root/.axon_site/_ro/trn_rl_repo"):
    if os.path.isdir(_p) and _p not in sys.path:
        sys.path.append(_p)

import numpy as np
import ml_dtypes

import bass_rust
import concourse.bass as bass
import concourse.tile as tile
from concourse import mybir
from concourse.bass_utils import run_bass_kernel_spmd

BF16 = ml_dtypes.bfloat16
F32 = mybir.dt.float32
F32R = mybir.dt.float32r
BF = mybir.dt.bfloat16

N = 4096          # nodes
NCORES = 8
R = N // NCORES   # rows (queries) per core
CJ = N // 128     # 32 key chunks
FIN = 512         # input feature dim of both layers
NF = FIN // 128   # fin chunks
NB = 13           # key chunks routed to path B (VectorE-only); rest path A
GRP = 4           # chunk-group size for fused VectorE ops

CORE_IDS = list(range(NCORES))

LAST_PERF = {}


# ---------------------------------------------------------------------------
# walrus workaround: it rejects instructions carrying >1 sync-wait command
# ("Too many sync wait commands").  Move excess waits onto preceding
# same-engine NoOps -- semantically identical (same-engine waits are totally
# ordered before the instruction).
def _split_excess_waits(nc, max_waits: int = 1) -> int:
    n_split = 0
    for fn in nc.m.functions:
        for bb in fn.blocks:
            insts = bb.instructions
            new_insts = []
            changed = False
            for ins in insts:
                si = ins.sync_info
                waits = list(si.on_wait) if si is not None else []
                if len(waits) > max_waits:
                    extra, keep = waits[:-max_waits], waits[-max_waits:]
                    for k in range(0, len(extra), max_waits):
                        chunk = extra[k : k + max_waits]
                        nop = bass_rust.InstNoOp(
                            name=f"{ins.name}-wsplit{k}", ins=[], outs=[]
                        )
                        nop.engine = ins.engine
                        nop.sync_info = mybir.SyncInfo(on_wait=chunk, on_update=[])
                        new_insts.append(nop)
                        n_split += 1
                    si.on_wait = keep
                    changed = True
                new_insts.append(ins)
            if changed:
                bb.instructions = new_insts
    return n_split


# ---------------------------------------------------------------------------
def _build_layer(H: int, HID: int, wh_on_device: bool = False):
    """One GAT layer, per-core program.

    Inputs (per core):
      xT     [FIN, N]       f32r  node features, transposed (layer 1 only)
      Wc     [FIN, H*HID]   f32r  weights, heads concatenated (layer 1 only)
      whxin  [128, CJ, H, WPH] bf16  precomputed Whx (layer 2 only)
      maskM  [N, R]         bf16  per-chunk mask: chunks < NB multiplicative
                                  0/1, chunks >= NB additive 0/-1000
      fsrcB  [H, 128, R]    bf16  f_src of this core's rows (bcast) (path A)
      uB     [H, 128, R]    bf16  exp(f_src) bcast (path B)
      u2B    [H, 128, R]    bf16  exp(0.2 f_src) bcast (path B)
      fdst   [128, H*CJ]    f32   f_dst, [p, h*CJ+c] = f_dst[h, 128c+p]
      fdst2  [128, H*CJ]    f32   0.2 * fdst
      vcol   [128, H*CJ]    bf16  exp(f_dst)
      v2col  [128, H*CJ]    bf16  exp(0.2 f_dst)
    Output:
      agg    [H, HID+1, R]  f32   rows 0..HID-1: unnormalized att @ Wh
                                  (transposed); row HID: softmax denominator
    """
    HD = H * HID
    WPH = HID + 2  # per-head stride in Whx: HID cols + ones col + pad

    nc = bass.Bass("TRN2", debug=False, num_devices=NCORES)
    whxin = nc.dram_tensor("whxin", [128, CJ, H, WPH], BF, kind="ExternalInput")
    maskM = nc.dram_tensor("maskM", [128, CJ, R], BF, kind="ExternalInput")
    fsrcB = nc.dram_tensor("fsrcB", [128, H, R], BF, kind="ExternalInput")
    uB = nc.dram_tensor("uB", [128, H, R], BF, kind="ExternalInput")
    u2B = nc.dram_tensor("u2B", [128, H, R], BF, kind="ExternalInput")
    fdst2 = nc.dram_tensor("fdst2", [128, H * CJ], F32, kind="ExternalInput")
    wcol = nc.dram_tensor("wcol", [128, H * CJ], F32, kind="ExternalInput")
    agg = nc.dram_tensor("agg", [H, HID + 1, R], F32, kind="ExternalOutput")

    EXP = mybir.ActivationFunctionType.Exp
    ADD = mybir.AluOpType.add
    MAX = mybir.AluOpType.max
    MUL = mybir.AluOpType.mult

    with tile.TileContext(nc) as tc, ExitStack() as ctx:
        cpool = ctx.enter_context(tc.tile_pool(name="const", bufs=1))
        wpool = ctx.enter_context(tc.tile_pool(name="whx", bufs=1))
        tpool = ctx.enter_context(tc.tile_pool(name="work", bufs=3))
        opool = ctx.enter_context(tc.tile_pool(name="out", bufs=2))
        paq = ctx.enter_context(tc.tile_pool(name="psa", bufs=3, space="PSUM"))

        # ---- resident constants -------------------------------------------
        # issue order matters: the small vectors and the first mask part feed
        # the first attention tiles; the remaining mask parts stream behind.
        u_t = cpool.tile([128, H, R], BF, tag="u")
        nc.sync.dma_start(u_t[:], uB[:])
        u2_t = cpool.tile([128, H, R], BF, tag="u2")
        nc.sync.dma_start(u2_t[:], u2B[:])
        w_t = cpool.tile([128, H * CJ], F32, tag="wcol")
        nc.sync.dma_start(w_t[:], wcol[:])
        fdst2_t = cpool.tile([128, H * CJ], F32, tag="fdst2")
        nc.sync.dma_start(fdst2_t[:], fdst2[:])
        fsrc_t = cpool.tile([128, H, R], BF, tag="fsrc")
        nc.sync.dma_start(fsrc_t[:], fsrcB[:])
        mask_t = cpool.tile([128, CJ, R], BF, tag="mask")

        # ---- phase 1: Whx[c] = [x @ Wc](chunk c) in bf16, + ones column ---
        whx = [None] * CJ

        def _load_whx():
            for c in range(CJ):
                wx = wpool.tile([128, H, WPH], BF, tag=f"whx{c}", name=f"whx{c}")
                nc.sync.dma_start(wx[:], whxin[:, c])
                whx[c] = wx

        # layer 2's whxin is tiny (147 KB) -- land it before the 4 MB mask so
        # the first aggregation matmul unblocks early; layer 1's is 4.3 MB and
        # must not delay the mask stream.
        if H == 1:
            _load_whx()
        NMQ = 8
        for mq in range(NMQ):
            cs = slice(mq * (CJ // NMQ), (mq + 1) * (CJ // NMQ))
            nc.sync.dma_start(mask_t[:, cs, :], maskM[:, cs, :])
        if H != 1:
            _load_whx()

        def _bcast(ap2d, G):
            return ap2d.rearrange("p (o r) -> p o r", o=1).broadcast_to((128, G, R))

        bgrps, agrps = [], []
        for lo, hi in ((0, NB), (NB, CJ)):
            c = lo
            while c < hi:
                g = min(GRP, hi - c)
                (bgrps if lo == 0 else agrps).append((c, g, lo == 0))
                c += g
        # interleave path-B (VectorE) and path-A (ScalarE) groups so the two
        # engines always have independent work in flight
        groups = []
        for i in range(max(len(bgrps), len(agrps))):
            if i < len(bgrps):
                groups.append(bgrps[i])
            if i < len(agrps):
                groups.append(agrps[i])

        # ---- phase 2: attention + aggregation -----------------------------
        for h in range(H):
            pa = paq.tile([HID + 1, R], F32, tag="psa")
            for gi, (c0, G, is_b) in enumerate(groups):
                p3p = tpool.tile([128, GRP, R], BF, tag="p3")
                if is_b:
                    # path B (VectorE only), with v = exp(f_dst) folded into
                    # the host-side Whx: p' = max(u, u2*w) * mask01,
                    # w = exp(-0.8 f_dst)
                    q2p = tpool.tile([128, GRP, R], BF, tag="q2")
                    for k in range(G):
                        o_ix = h * CJ + c0 + k
                        nc.vector.tensor_scalar(
                            q2p[:, k, :], u2_t[:, h, :],
                            w_t[:, o_ix : o_ix + 1], None, op0=MUL,
                        )
                    m0p = tpool.tile([128, GRP, R], BF, tag="m0")
                    nc.vector.tensor_tensor(
                        m0p[:, 0:G, :], q2p[:, 0:G, :],
                        _bcast(u_t[:, h, :], G), op=MAX,
                    )
                    nc.vector.tensor_tensor(
                        p3p[:, 0:G, :], m0p[:, 0:G, :],
                        mask_t[:, c0 : c0 + G, :], op=MUL,
                    )
                else:
                    # path A (ScalarE-heavy): p = max(exp(t0+fd), exp(.2 t0+.2 fd))
                    t0p = tpool.tile([128, GRP, R], BF, tag="t0")
                    nc.vector.tensor_tensor(
                        t0p[:, 0:G, :], mask_t[:, c0 : c0 + G, :],
                        _bcast(fsrc_t[:, h, :], G), op=ADD,
                    )
                    p1p = tpool.tile([128, GRP, R], BF, tag="p1")
                    p2p = tpool.tile([128, GRP, R], BF, tag="p2")
                    # bias-free first branch: one ScalarE op for the group
                    nc.scalar.activation(
                        p1p[:, 0:G, :], t0p[:, 0:G, :], EXP, scale=1.0,
                    )
                    for k in range(G):
                        o_ix = h * CJ + c0 + k
                        nc.scalar.activation(
                            p2p[:, k, :], t0p[:, k, :], EXP,
                            bias=fdst2_t[:, o_ix : o_ix + 1], scale=0.2,
                        )
                    nc.vector.tensor_tensor(
                        p3p[:, 0:G, :], p1p[:, 0:G, :], p2p[:, 0:G, :], op=MAX
                    )
                for k in range(G):
                    c = c0 + k
                    nc.tensor.matmul(
                        pa[:], whx[c][:, h, 0 : HID + 1], p3p[:, k, :],
                        start=(gi == 0 and k == 0),
                        stop=(gi == len(groups) - 1 and k == G - 1),
                    )
            o = opool.tile([HID + 1, R], F32, tag="aggo")
            nc.scalar.copy(o[:], pa[:])
            nc.sync.dma_start(agg[h], o[:])

    return nc


_PROGS = {}


def _get_prog(H, HID):
    """Build (and cache) the layer program with the walrus wait-split fix
    applied.  The fix is HW-only: CoreSim's event loop rejects the injected
    NoOps, so sim users should call _build_layer directly."""
    key = (H, HID)
    if key not in _PROGS:
        nc = _build_layer(H, HID)
        _split_excess_waits(nc)
        _PROGS[key] = nc
    return _PROGS[key]


def _elu(v):
    return np.where(v > 0, v, np.expm1(np.minimum(v, 0.0))).astype(np.float32)


def _host_inputs(f_src, f_dst, adj, Wh, H):
    """Shared per-layer host prep.  f_src/f_dst [N, H] f32, adj [N, N] i32,
    Wh [N, H*HID] f32 (pre-activation per-head features)."""
    HID = Wh.shape[1] // H
    WPH = HID + 2
    fdst_arr = np.ascontiguousarray(
        f_dst.T.reshape(H, CJ, 128).transpose(2, 0, 1).reshape(128, H * CJ)
    ).astype(np.float32)
    fdst2_arr = (-0.8 * fdst_arr).astype(np.float32)   # Exp-2 bias
    w_arr = np.exp(fdst2_arr).astype(np.float32)       # exp(-0.8 f_dst)

    # v = exp(f_dst) folded into the stationary operand; ones-col becomes v
    ev = np.exp(f_dst).astype(np.float32)  # [N, H]
    whx = np.zeros((128, CJ, H, WPH), np.float32)
    whx[:, :, :, :HID] = (
        (Wh.reshape(N, H, HID) * ev[:, :, None])
        .reshape(CJ, 128, H, HID).transpose(1, 0, 2, 3)
    )
    whx[:, :, :, HID] = ev.reshape(CJ, 128, H).transpose(1, 0, 2)

    shared = {
        "fdst2": fdst2_arr,
        "wcol": w_arr,
        "whxin": whx.astype(BF16),
    }
    per_core = []
    for i in range(NCORES):
        rows = slice(R * i, R * (i + 1))
        adjT = adj[rows, :].T.astype(np.float32)  # [N, R]
        mm = np.empty((N, R), np.float32)
        nb_rows = NB * 128
        mm[:nb_rows] = adjT[:nb_rows]                      # 0/1 multiplicative
        mm[nb_rows:] = (adjT[nb_rows:] - 1.0) * 1000.0     # 0/-1000 additive
        fs = np.ascontiguousarray(f_src[rows, :].T)  # [H, R]
        d = dict(shared)
        d["maskM"] = np.ascontiguousarray(
            mm.reshape(CJ, 128, R).transpose(1, 0, 2)
        ).astype(BF16)
        d["fsrcB"] = np.broadcast_to(fs[None, :, :], (128, H, R)).astype(BF16)
        d["uB"] = np.broadcast_to(
            np.exp(fs)[None, :, :], (128, H, R)
        ).astype(BF16)
        d["u2B"] = np.broadcast_to(
            np.exp(0.2 * fs)[None, :, :], (128, H, R)
        ).astype(BF16)
        per_core.append(d)
    return per_core


def _run_layer(nc, in_maps, H, HID, tag):
    t0 = time.time()
    res = run_bass_kernel_spmd(nc, in_maps, core_ids=CORE_IDS)
    LAST_PERF[f"{tag}_wall_s"] = time.time() - t0
    LAST_PERF[f"{tag}_exec_ns"] = res.exec_time_ns

    hT = np.empty((H * HID, N), np.float32)
    for i in range(NCORES):
        a = res.results[i]["agg"]  # [H, HID+1, R]
        denom = a[:, HID : HID + 1, :]
        hT[:, R * i : R * (i + 1)] = (a[:, :HID, :] / denom).reshape(H * HID, R)
    return hT


def kernel(x, adj, W1, a1, W2, a2):
    x = np.asarray(x, np.float32)
    adj = np.asarray(adj, np.int32)
    W1 = np.asarray(W1, np.float32)
    a1 = np.asarray(a1, np.float32)
    W2 = np.asarray(W2, np.float32)
    a2 = np.asarray(a2, np.float32)

    H1, HID1, OUT = W1.shape[0], W1.shape[2], W2.shape[1]

    progA = _get_prog(H1, HID1)
    progB = _get_prog(1, OUT)

    # ---- layer 1 ----------------------------------------------------------
    W1c = np.ascontiguousarray(W1.transpose(1, 0, 2).reshape(FIN, H1 * HID1))
    wsrc1 = np.einsum("hfk,hk->fh", W1, a1[:, :HID1, 0]).astype(np.float32)
    wdst1 = np.einsum("hfk,hk->fh", W1, a1[:, HID1:, 0]).astype(np.float32)
    f_src1 = x @ wsrc1  # [N, H]
    f_dst1 = x @ wdst1
    Wh1 = x @ W1c  # [N, H1*HID1]

    in_maps = _host_inputs(f_src1, f_dst1, adj, Wh1, H1)
    hT = _run_layer(progA, in_maps, H1, HID1, "layer1")
    hcatT = _elu(hT)  # [512, N] == h_cat.T (concat=True applies elu)

    # ---- layer 2 ----------------------------------------------------------
    hcat = np.ascontiguousarray(hcatT.T)  # [N, 512]
    wsrc2 = (W2 @ a2[:OUT, 0]).astype(np.float32)[:, None]
    wdst2 = (W2 @ a2[OUT:, 0]).astype(np.float32)[:, None]
    f_src2 = hcat @ wsrc2  # [N, 1]
    f_dst2 = hcat @ wdst2
    Wh2 = hcat @ W2  # [N, OUT]
    in_maps2 = _host_inputs(f_src2, f_dst2, adj, Wh2, 1)
    outT = _run_layer(progB, in_maps2, 1, OUT, "layer2")
    # layer 2: concat=False -> no inner elu; final output = elu(out)
    return np.ascontiguousarray(_elu(outT).T)


# revision 26
# speedup vs baseline: 1.0705x; 1.0705x over previous
"""Trainium2 Bass kernel for a 2-layer dense-adjacency GAT (nn_GAT_17824114278677).

Sharding: nodes (rows of the attention matrix) are sharded across the 8
NeuronCores, 512 rows per core; weights and node features are replicated.
Two SPMD launches (one per GAT layer) with a host-side gather of the layer-1
output in between.

Per-core dataflow: attention tiles are computed TRANSPOSED, [j=128
partitions, r=512 rows], so the aggregation att @ Wh maps directly onto the
PE (contraction over j on partitions) with zero on-chip transposes.
Identities used:

  * softmax is shift-invariant and all logits are bounded (|e| < ~6), so no
    max-subtraction is needed; masked logits get -1000 added (or a 0/1
    multiplicative mask), which produces exactly 0 after exp, matching the
    reference's -9e15 mask.
  * exp(leaky_relu(t)) == max(exp(t), exp(0.2 t)).  Two ways to evaluate it
    per 128x512 tile, assigned per key-chunk to balance ScalarE vs VectorE:
      path A (ScalarE-heavy): t0 = mask + f_src (DVE); exp(t0 + f_dst) and
        exp(0.2 t0 + 0.2 f_dst) on ScalarE (bias = per-partition AP);
        max on DVE.
      path B (VectorE-only, rank-1): with v = exp(f_dst) folded into the
        host-side Whx (and the ones-column replaced by v), the tile is
        p' = max(u, u2*w) * mask01 with u = exp(f_src) broadcast and
        w = exp(-0.8 f_dst) per-partition: one tensor_scalar + two fused
        tensor_tensor ops, all bf16.
  * softmax denominators ride along as a ones-column in the stationary
    operand; division + ELU happen on the host on the tiny per-head
    [HID+1, 512] outputs.

Wh = x @ W (0.4%% of the FLOPs) plus the per-node attention vectors
f_src/f_dst are computed on the host in fp32 and shipped pre-rounded to
bf16; all on-device attention/aggregation math runs in bf16 with fp32 PSUM
accumulation.  Measured on 8 axon-tunneled trn2 cores: ~211 us (layer 1) +
~52 us (layer 2) = ~263 us, end-to-end relative error vs the fp32 jax
reference ~7e-4.  VectorE/ScalarE both measure >93%% busy -- the kernel sits
at the elementwise-engine saturation floor of this op chain.
"""

import os
import sys
import time
from contextlib import ExitStack

for _p in ("/opt/trn_rl_repo", "/root/.axon_site/_ro/trn_rl_repo"):
    if os.path.isdir(_p) and _p not in sys.path:
        sys.path.append(_p)

import numpy as np
import ml_dtypes

import bass_rust
import concourse.bass as bass
import concourse.tile as tile
from concourse import mybir
from concourse.bass_utils import run_bass_kernel_spmd

BF16 = ml_dtypes.bfloat16
F32 = mybir.dt.float32
F32R = mybir.dt.float32r
BF = mybir.dt.bfloat16

N = 4096          # nodes
NCORES = 8
R = N // NCORES   # rows (queries) per core
CJ = N // 128     # 32 key chunks
FIN = 512         # input feature dim of both layers
NF = FIN // 128   # fin chunks
NB = 13           # key chunks routed to path B (VectorE-only); rest path A
GRP = 4           # chunk-group size for fused VectorE ops

CORE_IDS = list(range(NCORES))

LAST_PERF = {}


# ---------------------------------------------------------------------------
# walrus workaround: it rejects instructions carrying >1 sync-wait command
# ("Too many sync wait commands").  Move excess waits onto preceding
# same-engine NoOps -- semantically identical (same-engine waits are totally
# ordered before the instruction).
def _split_excess_waits(nc, max_waits: int = 1) -> int:
    n_split = 0
    for fn in nc.m.functions:
        for bb in fn.blocks:
            insts = bb.instructions
            new_insts = []
            changed = False
            for ins in insts:
                si = ins.sync_info
                waits = list(si.on_wait) if si is not None else []
                if len(waits) > max_waits:
                    extra, keep = waits[:-max_waits], waits[-max_waits:]
                    for k in range(0, len(extra), max_waits):
                        chunk = extra[k : k + max_waits]
                        nop = bass_rust.InstNoOp(
                            name=f"{ins.name}-wsplit{k}", ins=[], outs=[]
                        )
                        nop.engine = ins.engine
                        nop.sync_info = mybir.SyncInfo(on_wait=chunk, on_update=[])
                        new_insts.append(nop)
                        n_split += 1
                    si.on_wait = keep
                    changed = True
                new_insts.append(ins)
            if changed:
                bb.instructions = new_insts
    return n_split


# ---------------------------------------------------------------------------
def _build_layer(H: int, HID: int, wh_on_device: bool = False):
    """One GAT layer, per-core program.

    Inputs (per core):
      xT     [FIN, N]       f32r  node features, transposed (layer 1 only)
      Wc     [FIN, H*HID]   f32r  weights, heads concatenated (layer 1 only)
      whxin  [128, CJ, H, WPH] bf16  precomputed Whx (layer 2 only)
      maskM  [N, R]         bf16  per-chunk mask: chunks < NB multiplicative
                                  0/1, chunks >= NB additive 0/-1000
      fsrcB  [H, 128, R]    bf16  f_src of this core's rows (bcast) (path A)
      uB     [H, 128, R]    bf16  exp(f_src) bcast (path B)
      u2B    [H, 128, R]    bf16  exp(0.2 f_src) bcast (path B)
      fdst   [128, H*CJ]    f32   f_dst, [p, h*CJ+c] = f_dst[h, 128c+p]
      fdst2  [128, H*CJ]    f32   0.2 * fdst
      vcol   [128, H*CJ]    bf16  exp(f_dst)
      v2col  [128, H*CJ]    bf16  exp(0.2 f_dst)
    Output:
      agg    [H, HID+1, R]  f32   rows 0..HID-1: unnormalized att @ Wh
                                  (transposed); row HID: softmax denominator
    """
    HD = H * HID
    WPH = HID + 2  # per-head stride in Whx: HID cols + ones col + pad

    nc = bass.Bass("TRN2", debug=False, num_devices=NCORES)
    whxin = nc.dram_tensor("whxin", [128, CJ, H, WPH], BF, kind="ExternalInput")
    maskM = nc.dram_tensor("maskM", [128, CJ, R], BF, kind="ExternalInput")
    fsrcB = nc.dram_tensor("fsrcB", [128, H, R], BF, kind="ExternalInput")
    uB = nc.dram_tensor("uB", [128, H, R], BF, kind="ExternalInput")
    u2B = nc.dram_tensor("u2B", [128, H, R], BF, kind="ExternalInput")
    fdst2 = nc.dram_tensor("fdst2", [128, H * CJ], F32, kind="ExternalInput")
    wcol = nc.dram_tensor("wcol", [128, H * CJ], F32, kind="ExternalInput")
    agg = nc.dram_tensor("agg", [H, HID + 1, R], F32, kind="ExternalOutput")

    EXP = mybir.ActivationFunctionType.Exp
    ADD = mybir.AluOpType.add
    MAX = mybir.AluOpType.max
    MUL = mybir.AluOpType.mult

    with tile.TileContext(nc) as tc, ExitStack() as ctx:
        cpool = ctx.enter_context(tc.tile_pool(name="const", bufs=1))
        wpool = ctx.enter_context(tc.tile_pool(name="whx", bufs=1))
        tpool = ctx.enter_context(tc.tile_pool(name="work", bufs=3))
        opool = ctx.enter_context(tc.tile_pool(name="out", bufs=2))
        paq = ctx.enter_context(tc.tile_pool(name="psa", bufs=3, space="PSUM"))

        # ---- resident constants -------------------------------------------
        # issue order matters: the small vectors and the first mask part feed
        # the first attention tiles; the remaining mask parts stream behind.
        u_t = cpool.tile([128, H, R], BF, tag="u")
        nc.sync.dma_start(u_t[:], uB[:])
        u2_t = cpool.tile([128, H, R], BF, tag="u2")
        nc.sync.dma_start(u2_t[:], u2B[:])
        w_t = cpool.tile([128, H * CJ], F32, tag="wcol")
        nc.sync.dma_start(w_t[:], wcol[:])
        fdst2_t = cpool.tile([128, H * CJ], F32, tag="fdst2")
        nc.sync.dma_start(fdst2_t[:], fdst2[:])
        fsrc_t = cpool.tile([128, H, R], BF, tag="fsrc")
        nc.sync.dma_start(fsrc_t[:], fsrcB[:])
        mask_t = cpool.tile([128, CJ, R], BF, tag="mask")

        # ---- phase 1: Whx[c] = [x @ Wc](chunk c) in bf16, + ones column ---
        # layer 2's whxin is tiny (147 KB): land it as ONE DMA before the
        # 4 MB mask so the first aggregation matmul unblocks early.  Layer 1's
        # is 4.3 MB: keep per-chunk tiles streamed after the mask.
        if H == 1:
            whx_all = cpool.tile([128, CJ, H, WPH], BF, tag="whxall")
            nc.sync.dma_start(whx_all[:], whxin[:])
            whx = [whx_all[:, c] for c in range(CJ)]
        NMQ = 8
        for mq in range(NMQ):
            cs = slice(mq * (CJ // NMQ), (mq + 1) * (CJ // NMQ))
            nc.sync.dma_start(mask_t[:, cs, :], maskM[:, cs, :])
        if H != 1:
            whx = []
            for c in range(CJ):
                wx = wpool.tile([128, H, WPH], BF, tag=f"whx{c}", name=f"whx{c}")
                nc.sync.dma_start(wx[:], whxin[:, c])
                whx.append(wx)

        def _bcast(ap2d, G):
            return ap2d.rearrange("p (o r) -> p o r", o=1).broadcast_to((128, G, R))

        bgrps, agrps = [], []
        for lo, hi in ((0, NB), (NB, CJ)):
            c = lo
            while c < hi:
                g = min(GRP, hi - c)
                (bgrps if lo == 0 else agrps).append((c, g, lo == 0))
                c += g
        # interleave path-B (VectorE) and path-A (ScalarE) groups so the two
        # engines always have independent work in flight
        groups = []
        for i in range(max(len(bgrps), len(agrps))):
            if i < len(bgrps):
                groups.append(bgrps[i])
            if i < len(agrps):
                groups.append(agrps[i])

        # ---- phase 2: attention + aggregation -----------------------------
        for h in range(H):
            pa = paq.tile([HID + 1, R], F32, tag="psa")
            for gi, (c0, G, is_b) in enumerate(groups):
                p3p = tpool.tile([128, GRP, R], BF, tag="p3")
                if is_b:
                    # path B (VectorE only), with v = exp(f_dst) folded into
                    # the host-side Whx: p' = max(u, u2*w) * mask01,
                    # w = exp(-0.8 f_dst)
                    q2p = tpool.tile([128, GRP, R], BF, tag="q2")
                    for k in range(G):
                        o_ix = h * CJ + c0 + k
                        nc.vector.tensor_scalar(
                            q2p[:, k, :], u2_t[:, h, :],
                            w_t[:, o_ix : o_ix + 1], None, op0=MUL,
                        )
                    m0p = tpool.tile([128, GRP, R], BF, tag="m0")
                    nc.vector.tensor_tensor(
                        m0p[:, 0:G, :], q2p[:, 0:G, :],
                        _bcast(u_t[:, h, :], G), op=MAX,
                    )
                    nc.vector.tensor_tensor(
                        p3p[:, 0:G, :], m0p[:, 0:G, :],
                        mask_t[:, c0 : c0 + G, :], op=MUL,
                    )
                else:
                    # path A (ScalarE-heavy): p = max(exp(t0+fd), exp(.2 t0+.2 fd))
                    t0p = tpool.tile([128, GRP, R], BF, tag="t0")
                    nc.vector.tensor_tensor(
                        t0p[:, 0:G, :], mask_t[:, c0 : c0 + G, :],
                        _bcast(fsrc_t[:, h, :], G), op=ADD,
                    )
                    p1p = tpool.tile([128, GRP, R], BF, tag="p1")
                    p2p = tpool.tile([128, GRP, R], BF, tag="p2")
                    # bias-free first branch: one ScalarE op for the group
                    nc.scalar.activation(
                        p1p[:, 0:G, :], t0p[:, 0:G, :], EXP, scale=1.0,
                    )
                    for k in range(G):
                        o_ix = h * CJ + c0 + k
                        nc.scalar.activation(
                            p2p[:, k, :], t0p[:, k, :], EXP,
                            bias=fdst2_t[:, o_ix : o_ix + 1], scale=0.2,
                        )
                    nc.vector.tensor_tensor(
                        p3p[:, 0:G, :], p1p[:, 0:G, :], p2p[:, 0:G, :], op=MAX
                    )
                for k in range(G):
                    c = c0 + k
                    nc.tensor.matmul(
                        pa[:], whx[c][:, h, 0 : HID + 1], p3p[:, k, :],
                        start=(gi == 0 and k == 0),
                        stop=(gi == len(groups) - 1 and k == G - 1),
                    )
            o = opool.tile([HID + 1, R], F32, tag="aggo")
            nc.scalar.copy(o[:], pa[:])
            nc.sync.dma_start(agg[h], o[:])

    return nc


_PROGS = {}


def _get_prog(H, HID):
    """Build (and cache) the layer program with the walrus wait-split fix
    applied.  The fix is HW-only: CoreSim's event loop rejects the injected
    NoOps, so sim users should call _build_layer directly."""
    key = (H, HID)
    if key not in _PROGS:
        nc = _build_layer(H, HID)
        _split_excess_waits(nc)
        _PROGS[key] = nc
    return _PROGS[key]


def _elu(v):
    return np.where(v > 0, v, np.expm1(np.minimum(v, 0.0))).astype(np.float32)


def _host_inputs(f_src, f_dst, adj, Wh, H):
    """Shared per-layer host prep.  f_src/f_dst [N, H] f32, adj [N, N] i32,
    Wh [N, H*HID] f32 (pre-activation per-head features)."""
    HID = Wh.shape[1] // H
    WPH = HID + 2
    fdst_arr = np.ascontiguousarray(
        f_dst.T.reshape(H, CJ, 128).transpose(2, 0, 1).reshape(128, H * CJ)
    ).astype(np.float32)
    fdst2_arr = (-0.8 * fdst_arr).astype(np.float32)   # Exp-2 bias
    w_arr = np.exp(fdst2_arr).astype(np.float32)       # exp(-0.8 f_dst)

    # v = exp(f_dst) folded into the stationary operand; ones-col becomes v
    ev = np.exp(f_dst).astype(np.float32)  # [N, H]
    whx = np.zeros((128, CJ, H, WPH), np.float32)
    whx[:, :, :, :HID] = (
        (Wh.reshape(N, H, HID) * ev[:, :, None])
        .reshape(CJ, 128, H, HID).transpose(1, 0, 2, 3)
    )
    whx[:, :, :, HID] = ev.reshape(CJ, 128, H).transpose(1, 0, 2)

    shared = {
        "fdst2": fdst2_arr,
        "wcol": w_arr,
        "whxin": whx.astype(BF16),
    }
    per_core = []
    for i in range(NCORES):
        rows = slice(R * i, R * (i + 1))
        adjT = adj[rows, :].T.astype(np.float32)  # [N, R]
        mm = np.empty((N, R), np.float32)
        nb_rows = NB * 128
        mm[:nb_rows] = adjT[:nb_rows]                      # 0/1 multiplicative
        mm[nb_rows:] = (adjT[nb_rows:] - 1.0) * 1000.0     # 0/-1000 additive
        fs = np.ascontiguousarray(f_src[rows, :].T)  # [H, R]
        d = dict(shared)
        d["maskM"] = np.ascontiguousarray(
            mm.reshape(CJ, 128, R).transpose(1, 0, 2)
        ).astype(BF16)
        d["fsrcB"] = np.broadcast_to(fs[None, :, :], (128, H, R)).astype(BF16)
        d["uB"] = np.broadcast_to(
            np.exp(fs)[None, :, :], (128, H, R)
        ).astype(BF16)
        d["u2B"] = np.broadcast_to(
            np.exp(0.2 * fs)[None, :, :], (128, H, R)
        ).astype(BF16)
        per_core.append(d)
    return per_core


def _run_layer(nc, in_maps, H, HID, tag):
    t0 = time.time()
    res = run_bass_kernel_spmd(nc, in_maps, core_ids=CORE_IDS)
    LAST_PERF[f"{tag}_wall_s"] = time.time() - t0
    LAST_PERF[f"{tag}_exec_ns"] = res.exec_time_ns

    hT = np.empty((H * HID, N), np.float32)
    for i in range(NCORES):
        a = res.results[i]["agg"]  # [H, HID+1, R]
        denom = a[:, HID : HID + 1, :]
        hT[:, R * i : R * (i + 1)] = (a[:, :HID, :] / denom).reshape(H * HID, R)
    return hT


def kernel(x, adj, W1, a1, W2, a2):
    x = np.asarray(x, np.float32)
    adj = np.asarray(adj, np.int32)
    W1 = np.asarray(W1, np.float32)
    a1 = np.asarray(a1, np.float32)
    W2 = np.asarray(W2, np.float32)
    a2 = np.asarray(a2, np.float32)

    H1, HID1, OUT = W1.shape[0], W1.shape[2], W2.shape[1]

    progA = _get_prog(H1, HID1)
    progB = _get_prog(1, OUT)

    # ---- layer 1 ----------------------------------------------------------
    W1c = np.ascontiguousarray(W1.transpose(1, 0, 2).reshape(FIN, H1 * HID1))
    wsrc1 = np.einsum("hfk,hk->fh", W1, a1[:, :HID1, 0]).astype(np.float32)
    wdst1 = np.einsum("hfk,hk->fh", W1, a1[:, HID1:, 0]).astype(np.float32)
    f_src1 = x @ wsrc1  # [N, H]
    f_dst1 = x @ wdst1
    Wh1 = x @ W1c  # [N, H1*HID1]

    in_maps = _host_inputs(f_src1, f_dst1, adj, Wh1, H1)
    hT = _run_layer(progA, in_maps, H1, HID1, "layer1")
    hcatT = _elu(hT)  # [512, N] == h_cat.T (concat=True applies elu)

    # ---- layer 2 ----------------------------------------------------------
    hcat = np.ascontiguousarray(hcatT.T)  # [N, 512]
    wsrc2 = (W2 @ a2[:OUT, 0]).astype(np.float32)[:, None]
    wdst2 = (W2 @ a2[OUT:, 0]).astype(np.float32)[:, None]
    f_src2 = hcat @ wsrc2  # [N, 1]
    f_dst2 = hcat @ wdst2
    Wh2 = hcat @ W2  # [N, OUT]
    in_maps2 = _host_inputs(f_src2, f_dst2, adj, Wh2, 1)
    outT = _run_layer(progB, in_maps2, 1, OUT, "layer2")
    # layer 2: concat=False -> no inner elu; final output = elu(out)
    return np.ascontiguousarray(_elu(outT).T)


# revision 27
# speedup vs baseline: 1.0929x; 1.0210x over previous
"""Trainium2 Bass kernel for a 2-layer dense-adjacency GAT (nn_GAT_17824114278677).

Sharding: nodes (rows of the attention matrix) are sharded across the 8
NeuronCores, 512 rows per core; weights and node features are replicated.
Two SPMD launches (one per GAT layer) with a host-side gather of the layer-1
output in between.

Per-core dataflow: attention tiles are computed TRANSPOSED, [j=128
partitions, r=512 rows], so the aggregation att @ Wh maps directly onto the
PE (contraction over j on partitions) with zero on-chip transposes.
Identities used:

  * softmax is shift-invariant and all logits are bounded (|e| < ~6), so no
    max-subtraction is needed; masked logits get -1000 added (or a 0/1
    multiplicative mask), which produces exactly 0 after exp, matching the
    reference's -9e15 mask.
  * exp(leaky_relu(t)) == max(exp(t), exp(0.2 t)).  Two ways to evaluate it
    per 128x512 tile, assigned per key-chunk to balance ScalarE vs VectorE:
      path A (ScalarE-heavy): t0 = mask + f_src (DVE); exp(t0 + f_dst) and
        exp(0.2 t0 + 0.2 f_dst) on ScalarE (bias = per-partition AP);
        max on DVE.
      path B (VectorE-only, rank-1): with v = exp(f_dst) folded into the
        host-side Whx (and the ones-column replaced by v), the tile is
        p' = max(u, u2*w) * mask01 with u = exp(f_src) broadcast and
        w = exp(-0.8 f_dst) per-partition: one tensor_scalar + two fused
        tensor_tensor ops, all bf16.
  * softmax denominators ride along as a ones-column in the stationary
    operand; division + ELU happen on the host on the tiny per-head
    [HID+1, 512] outputs.

Wh = x @ W (0.4%% of the FLOPs) plus the per-node attention vectors
f_src/f_dst are computed on the host in fp32 and shipped pre-rounded to
bf16; all on-device attention/aggregation math runs in bf16 with fp32 PSUM
accumulation.  Measured on 8 axon-tunneled trn2 cores: ~211 us (layer 1) +
~52 us (layer 2) = ~263 us, end-to-end relative error vs the fp32 jax
reference ~7e-4.  VectorE/ScalarE both measure >93%% busy -- the kernel sits
at the elementwise-engine saturation floor of this op chain.
"""

import os
import sys
import time
from contextlib import ExitStack

for _p in ("/opt/trn_rl_repo", "/root/.axon_site/_ro/trn_rl_repo"):
    if os.path.isdir(_p) and _p not in sys.path:
        sys.path.append(_p)

import numpy as np
import ml_dtypes

import bass_rust
import concourse.bass as bass
import concourse.tile as tile
from concourse import mybir
from concourse.bass_utils import run_bass_kernel_spmd

BF16 = ml_dtypes.bfloat16
F32 = mybir.dt.float32
F32R = mybir.dt.float32r
BF = mybir.dt.bfloat16

N = 4096          # nodes
NCORES = 8
R = N // NCORES   # rows (queries) per core
CJ = N // 128     # 32 key chunks
FIN = 512         # input feature dim of both layers
NF = FIN // 128   # fin chunks
NB = 13           # key chunks routed to path B (VectorE-only); rest path A
GRP = 4           # chunk-group size for fused VectorE ops

CORE_IDS = list(range(NCORES))

LAST_PERF = {}


# ---------------------------------------------------------------------------
# walrus workaround: it rejects instructions carrying >1 sync-wait command
# ("Too many sync wait commands").  Move excess waits onto preceding
# same-engine NoOps -- semantically identical (same-engine waits are totally
# ordered before the instruction).
def _split_excess_waits(nc, max_waits: int = 1) -> int:
    n_split = 0
    for fn in nc.m.functions:
        for bb in fn.blocks:
            insts = bb.instructions
            new_insts = []
            changed = False
            for ins in insts:
                si = ins.sync_info
                waits = list(si.on_wait) if si is not None else []
                if len(waits) > max_waits:
                    extra, keep = waits[:-max_waits], waits[-max_waits:]
                    for k in range(0, len(extra), max_waits):
                        chunk = extra[k : k + max_waits]
                        nop = bass_rust.InstNoOp(
                            name=f"{ins.name}-wsplit{k}", ins=[], outs=[]
                        )
                        nop.engine = ins.engine
                        nop.sync_info = mybir.SyncInfo(on_wait=chunk, on_update=[])
                        new_insts.append(nop)
                        n_split += 1
                    si.on_wait = keep
                    changed = True
                new_insts.append(ins)
            if changed:
                bb.instructions = new_insts
    return n_split


# ---------------------------------------------------------------------------
def _build_layer(H: int, HID: int, wh_on_device: bool = False):
    """One GAT layer, per-core program.

    Inputs (per core):
      xT     [FIN, N]       f32r  node features, transposed (layer 1 only)
      Wc     [FIN, H*HID]   f32r  weights, heads concatenated (layer 1 only)
      whxin  [128, CJ, H, WPH] bf16  precomputed Whx (layer 2 only)
      maskM  [N, R]         bf16  per-chunk mask: chunks < NB multiplicative
                                  0/1, chunks >= NB additive 0/-1000
      fsrcB  [H, 128, R]    bf16  f_src of this core's rows (bcast) (path A)
      uB     [H, 128, R]    bf16  exp(f_src) bcast (path B)
      u2B    [H, 128, R]    bf16  exp(0.2 f_src) bcast (path B)
      fdst   [128, H*CJ]    f32   f_dst, [p, h*CJ+c] = f_dst[h, 128c+p]
      fdst2  [128, H*CJ]    f32   0.2 * fdst
      vcol   [128, H*CJ]    bf16  exp(f_dst)
      v2col  [128, H*CJ]    bf16  exp(0.2 f_dst)
    Output:
      agg    [H, HID+1, R]  f32   rows 0..HID-1: unnormalized att @ Wh
                                  (transposed); row HID: softmax denominator
    """
    HD = H * HID
    WPH = HID + 2  # per-head stride in Whx: HID cols + ones col + pad

    nc = bass.Bass("TRN2", debug=False, num_devices=NCORES)
    whxin = nc.dram_tensor("whxin", [128, CJ, H, WPH], BF, kind="ExternalInput")
    maskM = nc.dram_tensor("maskM", [128, CJ, R], BF, kind="ExternalInput")
    fsrcB = nc.dram_tensor("fsrcB", [128, H, R], BF, kind="ExternalInput")
    uB = nc.dram_tensor("uB", [128, H, R], BF, kind="ExternalInput")
    u2B = nc.dram_tensor("u2B", [128, H, R], BF, kind="ExternalInput")
    fdst2 = nc.dram_tensor("fdst2", [128, H * CJ], F32, kind="ExternalInput")
    wcol = nc.dram_tensor("wcol", [128, H * CJ], F32, kind="ExternalInput")
    agg = nc.dram_tensor("agg", [H, HID + 1, R], F32, kind="ExternalOutput")

    EXP = mybir.ActivationFunctionType.Exp
    ADD = mybir.AluOpType.add
    MAX = mybir.AluOpType.max
    MUL = mybir.AluOpType.mult

    with tile.TileContext(nc) as tc, ExitStack() as ctx:
        cpool = ctx.enter_context(tc.tile_pool(name="const", bufs=1))
        wpool = ctx.enter_context(tc.tile_pool(name="whx", bufs=1))
        tpool = ctx.enter_context(tc.tile_pool(name="work", bufs=3))
        opool = ctx.enter_context(tc.tile_pool(name="out", bufs=2))
        paq = ctx.enter_context(tc.tile_pool(name="psa", bufs=3, space="PSUM"))

        # ---- resident constants -------------------------------------------
        # issue order matters: the small vectors and the first mask part feed
        # the first attention tiles; the remaining mask parts stream behind.
        u_t = cpool.tile([128, H, R], BF, tag="u")
        nc.sync.dma_start(u_t[:], uB[:])
        u2_t = cpool.tile([128, H, R], BF, tag="u2")
        nc.sync.dma_start(u2_t[:], u2B[:])
        w_t = cpool.tile([128, H * CJ], F32, tag="wcol")
        nc.sync.dma_start(w_t[:], wcol[:])
        fdst2_t = cpool.tile([128, H * CJ], F32, tag="fdst2")
        nc.sync.dma_start(fdst2_t[:], fdst2[:])
        fsrc_t = cpool.tile([128, H, R], BF, tag="fsrc")
        nc.sync.dma_start(fsrc_t[:], fsrcB[:])
        mask_t = cpool.tile([128, CJ, R], BF, tag="mask")

        # ---- phase 1: Whx[c] = [x @ Wc](chunk c) in bf16, + ones column ---
        NMQ = 8
        for mq in range(NMQ):
            cs = slice(mq * (CJ // NMQ), (mq + 1) * (CJ // NMQ))
            nc.sync.dma_start(mask_t[:, cs, :], maskM[:, cs, :])

        whx = []
        for c in range(CJ):
            wx = wpool.tile([128, H, WPH], BF, tag=f"whx{c}", name=f"whx{c}")
            nc.sync.dma_start(wx[:], whxin[:, c])
            whx.append(wx)

        def _bcast(ap2d, G):
            return ap2d.rearrange("p (o r) -> p o r", o=1).broadcast_to((128, G, R))

        bgrps, agrps = [], []
        for lo, hi in ((0, NB), (NB, CJ)):
            c = lo
            while c < hi:
                g = min(GRP, hi - c)
                (bgrps if lo == 0 else agrps).append((c, g, lo == 0))
                c += g
        # interleave path-B (VectorE) and path-A (ScalarE) groups so the two
        # engines always have independent work in flight
        groups = []
        for i in range(max(len(bgrps), len(agrps))):
            if i < len(bgrps):
                groups.append(bgrps[i])
            if i < len(agrps):
                groups.append(agrps[i])

        # ---- phase 2: attention + aggregation -----------------------------
        for h in range(H):
            pa = paq.tile([HID + 1, R], F32, tag="psa")
            for gi, (c0, G, is_b) in enumerate(groups):
                p3p = tpool.tile([128, GRP, R], BF, tag="p3")
                if is_b:
                    # path B (VectorE only), with v = exp(f_dst) folded into
                    # the host-side Whx: p' = max(u, u2*w) * mask01,
                    # w = exp(-0.8 f_dst)
                    q2p = tpool.tile([128, GRP, R], BF, tag="q2")
                    for k in range(G):
                        o_ix = h * CJ + c0 + k
                        nc.vector.tensor_scalar(
                            q2p[:, k, :], u2_t[:, h, :],
                            w_t[:, o_ix : o_ix + 1], None, op0=MUL,
                        )
                    m0p = tpool.tile([128, GRP, R], BF, tag="m0")
                    nc.vector.tensor_tensor(
                        m0p[:, 0:G, :], q2p[:, 0:G, :],
                        _bcast(u_t[:, h, :], G), op=MAX,
                    )
                    nc.vector.tensor_tensor(
                        p3p[:, 0:G, :], m0p[:, 0:G, :],
                        mask_t[:, c0 : c0 + G, :], op=MUL,
                    )
                else:
                    # path A (ScalarE-heavy): p = max(exp(t0+fd), exp(.2 t0+.2 fd))
                    t0p = tpool.tile([128, GRP, R], BF, tag="t0")
                    nc.vector.tensor_tensor(
                        t0p[:, 0:G, :], mask_t[:, c0 : c0 + G, :],
                        _bcast(fsrc_t[:, h, :], G), op=ADD,
                    )
                    p1p = tpool.tile([128, GRP, R], BF, tag="p1")
                    p2p = tpool.tile([128, GRP, R], BF, tag="p2")
                    # bias-free first branch: one ScalarE op for the group
                    nc.scalar.activation(
                        p1p[:, 0:G, :], t0p[:, 0:G, :], EXP, scale=1.0,
                    )
                    for k in range(G):
                        o_ix = h * CJ + c0 + k
                        nc.scalar.activation(
                            p2p[:, k, :], t0p[:, k, :], EXP,
                            bias=fdst2_t[:, o_ix : o_ix + 1], scale=0.2,
                        )
                    nc.vector.tensor_tensor(
                        p3p[:, 0:G, :], p1p[:, 0:G, :], p2p[:, 0:G, :], op=MAX
                    )
                for k in range(G):
                    c = c0 + k
                    nc.tensor.matmul(
                        pa[:], whx[c][:, h, 0 : HID + 1], p3p[:, k, :],
                        start=(gi == 0 and k == 0),
                        stop=(gi == len(groups) - 1 and k == G - 1),
                    )
            o = opool.tile([HID + 1, R], F32, tag="aggo")
            nc.vector.tensor_copy(o[:], pa[:])
            nc.sync.dma_start(agg[h], o[:])

    return nc


_PROGS = {}


def _get_prog(H, HID):
    """Build (and cache) the layer program with the walrus wait-split fix
    applied.  The fix is HW-only: CoreSim's event loop rejects the injected
    NoOps, so sim users should call _build_layer directly."""
    key = (H, HID)
    if key not in _PROGS:
        nc = _build_layer(H, HID)
        _split_excess_waits(nc)
        _PROGS[key] = nc
    return _PROGS[key]


def _elu(v):
    return np.where(v > 0, v, np.expm1(np.minimum(v, 0.0))).astype(np.float32)


def _host_inputs(f_src, f_dst, adj, Wh, H):
    """Shared per-layer host prep.  f_src/f_dst [N, H] f32, adj [N, N] i32,
    Wh [N, H*HID] f32 (pre-activation per-head features)."""
    HID = Wh.shape[1] // H
    WPH = HID + 2
    fdst_arr = np.ascontiguousarray(
        f_dst.T.reshape(H, CJ, 128).transpose(2, 0, 1).reshape(128, H * CJ)
    ).astype(np.float32)
    fdst2_arr = (-0.8 * fdst_arr).astype(np.float32)   # Exp-2 bias
    w_arr = np.exp(fdst2_arr).astype(np.float32)       # exp(-0.8 f_dst)

    # v = exp(f_dst) folded into the stationary operand; ones-col becomes v
    ev = np.exp(f_dst).astype(np.float32)  # [N, H]
    whx = np.zeros((128, CJ, H, WPH), np.float32)
    whx[:, :, :, :HID] = (
        (Wh.reshape(N, H, HID) * ev[:, :, None])
        .reshape(CJ, 128, H, HID).transpose(1, 0, 2, 3)
    )
    whx[:, :, :, HID] = ev.reshape(CJ, 128, H).transpose(1, 0, 2)

    shared = {
        "fdst2": fdst2_arr,
        "wcol": w_arr,
        "whxin": whx.astype(BF16),
    }
    per_core = []
    for i in range(NCORES):
        rows = slice(R * i, R * (i + 1))
        adjT = adj[rows, :].T.astype(np.float32)  # [N, R]
        mm = np.empty((N, R), np.float32)
        nb_rows = NB * 128
        mm[:nb_rows] = adjT[:nb_rows]                      # 0/1 multiplicative
        mm[nb_rows:] = (adjT[nb_rows:] - 1.0) * 1000.0     # 0/-1000 additive
        fs = np.ascontiguousarray(f_src[rows, :].T)  # [H, R]
        d = dict(shared)
        d["maskM"] = np.ascontiguousarray(
            mm.reshape(CJ, 128, R).transpose(1, 0, 2)
        ).astype(BF16)
        d["fsrcB"] = np.broadcast_to(fs[None, :, :], (128, H, R)).astype(BF16)
        d["uB"] = np.broadcast_to(
            np.exp(fs)[None, :, :], (128, H, R)
        ).astype(BF16)
        d["u2B"] = np.broadcast_to(
            np.exp(0.2 * fs)[None, :, :], (128, H, R)
        ).astype(BF16)
        per_core.append(d)
    return per_core


def _run_layer(nc, in_maps, H, HID, tag):
    t0 = time.time()
    res = run_bass_kernel_spmd(nc, in_maps, core_ids=CORE_IDS)
    LAST_PERF[f"{tag}_wall_s"] = time.time() - t0
    LAST_PERF[f"{tag}_exec_ns"] = res.exec_time_ns

    hT = np.empty((H * HID, N), np.float32)
    for i in range(NCORES):
        a = res.results[i]["agg"]  # [H, HID+1, R]
        denom = a[:, HID : HID + 1, :]
        hT[:, R * i : R * (i + 1)] = (a[:, :HID, :] / denom).reshape(H * HID, R)
    return hT


def kernel(x, adj, W1, a1, W2, a2):
    x = np.asarray(x, np.float32)
    adj = np.asarray(adj, np.int32)
    W1 = np.asarray(W1, np.float32)
    a1 = np.asarray(a1, np.float32)
    W2 = np.asarray(W2, np.float32)
    a2 = np.asarray(a2, np.float32)

    H1, HID1, OUT = W1.shape[0], W1.shape[2], W2.shape[1]

    progA = _get_prog(H1, HID1)
    progB = _get_prog(1, OUT)

    # ---- layer 1 ----------------------------------------------------------
    W1c = np.ascontiguousarray(W1.transpose(1, 0, 2).reshape(FIN, H1 * HID1))
    wsrc1 = np.einsum("hfk,hk->fh", W1, a1[:, :HID1, 0]).astype(np.float32)
    wdst1 = np.einsum("hfk,hk->fh", W1, a1[:, HID1:, 0]).astype(np.float32)
    f_src1 = x @ wsrc1  # [N, H]
    f_dst1 = x @ wdst1
    Wh1 = x @ W1c  # [N, H1*HID1]

    in_maps = _host_inputs(f_src1, f_dst1, adj, Wh1, H1)
    hT = _run_layer(progA, in_maps, H1, HID1, "layer1")
    hcatT = _elu(hT)  # [512, N] == h_cat.T (concat=True applies elu)

    # ---- layer 2 ----------------------------------------------------------
    hcat = np.ascontiguousarray(hcatT.T)  # [N, 512]
    wsrc2 = (W2 @ a2[:OUT, 0]).astype(np.float32)[:, None]
    wdst2 = (W2 @ a2[OUT:, 0]).astype(np.float32)[:, None]
    f_src2 = hcat @ wsrc2  # [N, 1]
    f_dst2 = hcat @ wdst2
    Wh2 = hcat @ W2  # [N, OUT]
    in_maps2 = _host_inputs(f_src2, f_dst2, adj, Wh2, 1)
    outT = _run_layer(progB, in_maps2, 1, OUT, "layer2")
    # layer 2: concat=False -> no inner elu; final output = elu(out)
    return np.ascontiguousarray(_elu(outT).T)


# revision 28
# speedup vs baseline: 1.0964x; 1.0032x over previous
"""Trainium2 Bass kernel for a 2-layer dense-adjacency GAT (nn_GAT_17824114278677).

Sharding: nodes (rows of the attention matrix) are sharded across the 8
NeuronCores, 512 rows per core; weights and node features are replicated.
Two SPMD launches (one per GAT layer) with a host-side gather of the layer-1
output in between.

Per-core dataflow: attention tiles are computed TRANSPOSED, [j=128
partitions, r=512 rows], so the aggregation att @ Wh maps directly onto the
PE (contraction over j on partitions) with zero on-chip transposes.
Identities used:

  * softmax is shift-invariant and all logits are bounded (|e| < ~6), so no
    max-subtraction is needed; masked logits get -1000 added (or a 0/1
    multiplicative mask), which produces exactly 0 after exp, matching the
    reference's -9e15 mask.
  * exp(leaky_relu(t)) == max(exp(t), exp(0.2 t)).  Two ways to evaluate it
    per 128x512 tile, assigned per key-chunk to balance ScalarE vs VectorE:
      path A (ScalarE-heavy): t0 = mask + f_src (DVE); exp(t0 + f_dst) and
        exp(0.2 t0 + 0.2 f_dst) on ScalarE (bias = per-partition AP);
        max on DVE.
      path B (VectorE-only, rank-1): with v = exp(f_dst) folded into the
        host-side Whx (and the ones-column replaced by v), the tile is
        p' = max(u, u2*w) * mask01 with u = exp(f_src) broadcast and
        w = exp(-0.8 f_dst) per-partition: one tensor_scalar + two fused
        tensor_tensor ops, all bf16.
  * softmax denominators ride along as a ones-column in the stationary
    operand; division + ELU happen on the host on the tiny per-head
    [HID+1, 512] outputs.

Wh = x @ W (0.4%% of the FLOPs) plus the per-node attention vectors
f_src/f_dst are computed on the host in fp32 and shipped pre-rounded to
bf16; all on-device attention/aggregation math runs in bf16 with fp32 PSUM
accumulation.  Measured on 8 axon-tunneled trn2 cores: ~211 us (layer 1) +
~52 us (layer 2) = ~263 us, end-to-end relative error vs the fp32 jax
reference ~7e-4.  VectorE/ScalarE both measure >93%% busy -- the kernel sits
at the elementwise-engine saturation floor of this op chain.
"""

import os
import sys
import time
from contextlib import ExitStack

for _p in ("/opt/trn_rl_repo", "/root/.axon_site/_ro/trn_rl_repo"):
    if os.path.isdir(_p) and _p not in sys.path:
        sys.path.append(_p)

import numpy as np
import ml_dtypes

import bass_rust
import concourse.bass as bass
import concourse.tile as tile
from concourse import mybir
from concourse.bass_utils import run_bass_kernel_spmd

BF16 = ml_dtypes.bfloat16
F32 = mybir.dt.float32
F32R = mybir.dt.float32r
BF = mybir.dt.bfloat16

N = 4096          # nodes
NCORES = 8
R = N // NCORES   # rows (queries) per core
CJ = N // 128     # 32 key chunks
FIN = 512         # input feature dim of both layers
NF = FIN // 128   # fin chunks
NB = 13           # L1 key chunks routed to path B (VectorE-only); rest path A
NB2 = 16          # same for layer 2 (its ScalarE/VectorE balance differs)
GRP = 4           # chunk-group size for fused VectorE ops

CORE_IDS = list(range(NCORES))

LAST_PERF = {}


# ---------------------------------------------------------------------------
# walrus workaround: it rejects instructions carrying >1 sync-wait command
# ("Too many sync wait commands").  Move excess waits onto preceding
# same-engine NoOps -- semantically identical (same-engine waits are totally
# ordered before the instruction).
def _split_excess_waits(nc, max_waits: int = 1) -> int:
    n_split = 0
    for fn in nc.m.functions:
        for bb in fn.blocks:
            insts = bb.instructions
            new_insts = []
            changed = False
            for ins in insts:
                si = ins.sync_info
                waits = list(si.on_wait) if si is not None else []
                if len(waits) > max_waits:
                    extra, keep = waits[:-max_waits], waits[-max_waits:]
                    for k in range(0, len(extra), max_waits):
                        chunk = extra[k : k + max_waits]
                        nop = bass_rust.InstNoOp(
                            name=f"{ins.name}-wsplit{k}", ins=[], outs=[]
                        )
                        nop.engine = ins.engine
                        nop.sync_info = mybir.SyncInfo(on_wait=chunk, on_update=[])
                        new_insts.append(nop)
                        n_split += 1
                    si.on_wait = keep
                    changed = True
                new_insts.append(ins)
            if changed:
                bb.instructions = new_insts
    return n_split


# ---------------------------------------------------------------------------
def _build_layer(H: int, HID: int, nb: int = NB):
    """One GAT layer, per-core program.

    Inputs (per core):
      xT     [FIN, N]       f32r  node features, transposed (layer 1 only)
      Wc     [FIN, H*HID]   f32r  weights, heads concatenated (layer 1 only)
      whxin  [128, CJ, H, WPH] bf16  precomputed Whx (layer 2 only)
      maskM  [N, R]         bf16  per-chunk mask: chunks < NB multiplicative
                                  0/1, chunks >= NB additive 0/-1000
      fsrcB  [H, 128, R]    bf16  f_src of this core's rows (bcast) (path A)
      uB     [H, 128, R]    bf16  exp(f_src) bcast (path B)
      u2B    [H, 128, R]    bf16  exp(0.2 f_src) bcast (path B)
      fdst   [128, H*CJ]    f32   f_dst, [p, h*CJ+c] = f_dst[h, 128c+p]
      fdst2  [128, H*CJ]    f32   0.2 * fdst
      vcol   [128, H*CJ]    bf16  exp(f_dst)
      v2col  [128, H*CJ]    bf16  exp(0.2 f_dst)
    Output:
      agg    [H, HID+1, R]  f32   rows 0..HID-1: unnormalized att @ Wh
                                  (transposed); row HID: softmax denominator
    """
    HD = H * HID
    WPH = HID + 2  # per-head stride in Whx: HID cols + ones col + pad

    nc = bass.Bass("TRN2", debug=False, num_devices=NCORES)
    whxin = nc.dram_tensor("whxin", [128, CJ, H, WPH], BF, kind="ExternalInput")
    maskM = nc.dram_tensor("maskM", [128, CJ, R], BF, kind="ExternalInput")
    fsrcB = nc.dram_tensor("fsrcB", [128, H, R], BF, kind="ExternalInput")
    uB = nc.dram_tensor("uB", [128, H, R], BF, kind="ExternalInput")
    u2B = nc.dram_tensor("u2B", [128, H, R], BF, kind="ExternalInput")
    fdst2 = nc.dram_tensor("fdst2", [128, H * CJ], F32, kind="ExternalInput")
    wcol = nc.dram_tensor("wcol", [128, H * CJ], F32, kind="ExternalInput")
    agg = nc.dram_tensor("agg", [H, HID + 1, R], F32, kind="ExternalOutput")

    EXP = mybir.ActivationFunctionType.Exp
    ADD = mybir.AluOpType.add
    MAX = mybir.AluOpType.max
    MUL = mybir.AluOpType.mult

    with tile.TileContext(nc) as tc, ExitStack() as ctx:
        cpool = ctx.enter_context(tc.tile_pool(name="const", bufs=1))
        wpool = ctx.enter_context(tc.tile_pool(name="whx", bufs=1))
        tpool = ctx.enter_context(tc.tile_pool(name="work", bufs=3))
        opool = ctx.enter_context(tc.tile_pool(name="out", bufs=2))
        paq = ctx.enter_context(tc.tile_pool(name="psa", bufs=3, space="PSUM"))

        # ---- resident constants -------------------------------------------
        # issue order matters: the small vectors and the first mask part feed
        # the first attention tiles; the remaining mask parts stream behind.
        u_t = cpool.tile([128, H, R], BF, tag="u")
        nc.sync.dma_start(u_t[:], uB[:])
        u2_t = cpool.tile([128, H, R], BF, tag="u2")
        nc.sync.dma_start(u2_t[:], u2B[:])
        w_t = cpool.tile([128, H * CJ], F32, tag="wcol")
        nc.sync.dma_start(w_t[:], wcol[:])
        fdst2_t = cpool.tile([128, H * CJ], F32, tag="fdst2")
        nc.sync.dma_start(fdst2_t[:], fdst2[:])
        fsrc_t = cpool.tile([128, H, R], BF, tag="fsrc")
        nc.sync.dma_start(fsrc_t[:], fsrcB[:])
        mask_t = cpool.tile([128, CJ, R], BF, tag="mask")

        # ---- phase 1: Whx[c] = [x @ Wc](chunk c) in bf16, + ones column ---
        NMQ = 8
        for mq in range(NMQ):
            cs = slice(mq * (CJ // NMQ), (mq + 1) * (CJ // NMQ))
            nc.sync.dma_start(mask_t[:, cs, :], maskM[:, cs, :])

        whx = []
        for c in range(CJ):
            wx = wpool.tile([128, H, WPH], BF, tag=f"whx{c}", name=f"whx{c}")
            nc.sync.dma_start(wx[:], whxin[:, c])
            whx.append(wx)

        def _bcast(ap2d, G):
            return ap2d.rearrange("p (o r) -> p o r", o=1).broadcast_to((128, G, R))

        bgrps, agrps = [], []
        for lo, hi in ((0, nb), (nb, CJ)):
            c = lo
            while c < hi:
                g = min(GRP, hi - c)
                (bgrps if lo == 0 else agrps).append((c, g, lo == 0))
                c += g
        # interleave path-B (VectorE) and path-A (ScalarE) groups so the two
        # engines always have independent work in flight
        groups = []
        for i in range(max(len(bgrps), len(agrps))):
            if i < len(bgrps):
                groups.append(bgrps[i])
            if i < len(agrps):
                groups.append(agrps[i])

        # ---- phase 2: attention + aggregation -----------------------------
        for h in range(H):
            pa = paq.tile([HID + 1, R], F32, tag="psa")
            for gi, (c0, G, is_b) in enumerate(groups):
                p3p = tpool.tile([128, GRP, R], BF, tag="p3")
                if is_b:
                    # path B (VectorE only), with v = exp(f_dst) folded into
                    # the host-side Whx: p' = max(u, u2*w) * mask01,
                    # w = exp(-0.8 f_dst)
                    q2p = tpool.tile([128, GRP, R], BF, tag="q2")
                    for k in range(G):
                        o_ix = h * CJ + c0 + k
                        nc.vector.tensor_scalar(
                            q2p[:, k, :], u2_t[:, h, :],
                            w_t[:, o_ix : o_ix + 1], None, op0=MUL,
                        )
                    m0p = tpool.tile([128, GRP, R], BF, tag="m0")
                    nc.vector.tensor_tensor(
                        m0p[:, 0:G, :], q2p[:, 0:G, :],
                        _bcast(u_t[:, h, :], G), op=MAX,
                    )
                    nc.vector.tensor_tensor(
                        p3p[:, 0:G, :], m0p[:, 0:G, :],
                        mask_t[:, c0 : c0 + G, :], op=MUL,
                    )
                else:
                    # path A (ScalarE-heavy): p = max(exp(t0+fd), exp(.2 t0+.2 fd))
                    t0p = tpool.tile([128, GRP, R], BF, tag="t0")
                    nc.vector.tensor_tensor(
                        t0p[:, 0:G, :], mask_t[:, c0 : c0 + G, :],
                        _bcast(fsrc_t[:, h, :], G), op=ADD,
                    )
                    p1p = tpool.tile([128, GRP, R], BF, tag="p1")
                    p2p = tpool.tile([128, GRP, R], BF, tag="p2")
                    # bias-free first branch: one ScalarE op for the group
                    nc.scalar.activation(
                        p1p[:, 0:G, :], t0p[:, 0:G, :], EXP, scale=1.0,
                    )
                    for k in range(G):
                        o_ix = h * CJ + c0 + k
                        nc.scalar.activation(
                            p2p[:, k, :], t0p[:, k, :], EXP,
                            bias=fdst2_t[:, o_ix : o_ix + 1], scale=0.2,
                        )
                    nc.vector.tensor_tensor(
                        p3p[:, 0:G, :], p1p[:, 0:G, :], p2p[:, 0:G, :], op=MAX
                    )
                for k in range(G):
                    c = c0 + k
                    nc.tensor.matmul(
                        pa[:], whx[c][:, h, 0 : HID + 1], p3p[:, k, :],
                        start=(gi == 0 and k == 0),
                        stop=(gi == len(groups) - 1 and k == G - 1),
                    )
            o = opool.tile([HID + 1, R], F32, tag="aggo")
            nc.vector.tensor_copy(o[:], pa[:])
            nc.sync.dma_start(agg[h], o[:])

    return nc


_PROGS = {}


def _get_prog(H, HID, nb=NB):
    """Build (and cache) the layer program with the walrus wait-split fix
    applied.  The fix is HW-only: CoreSim's event loop rejects the injected
    NoOps, so sim users should call _build_layer directly."""
    key = (H, HID, nb)
    if key not in _PROGS:
        nc = _build_layer(H, HID, nb)
        _split_excess_waits(nc)
        _PROGS[key] = nc
    return _PROGS[key]


def _elu(v):
    return np.where(v > 0, v, np.expm1(np.minimum(v, 0.0))).astype(np.float32)


def _host_inputs(f_src, f_dst, adj, Wh, H, nb=NB):
    """Shared per-layer host prep.  f_src/f_dst [N, H] f32, adj [N, N] i32,
    Wh [N, H*HID] f32 (pre-activation per-head features)."""
    HID = Wh.shape[1] // H
    WPH = HID + 2
    fdst_arr = np.ascontiguousarray(
        f_dst.T.reshape(H, CJ, 128).transpose(2, 0, 1).reshape(128, H * CJ)
    ).astype(np.float32)
    fdst2_arr = (-0.8 * fdst_arr).astype(np.float32)   # Exp-2 bias
    w_arr = np.exp(fdst2_arr).astype(np.float32)       # exp(-0.8 f_dst)

    # v = exp(f_dst) folded into the stationary operand; ones-col becomes v
    ev = np.exp(f_dst).astype(np.float32)  # [N, H]
    whx = np.zeros((128, CJ, H, WPH), np.float32)
    whx[:, :, :, :HID] = (
        (Wh.reshape(N, H, HID) * ev[:, :, None])
        .reshape(CJ, 128, H, HID).transpose(1, 0, 2, 3)
    )
    whx[:, :, :, HID] = ev.reshape(CJ, 128, H).transpose(1, 0, 2)

    shared = {
        "fdst2": fdst2_arr,
        "wcol": w_arr,
        "whxin": whx.astype(BF16),
    }
    per_core = []
    for i in range(NCORES):
        rows = slice(R * i, R * (i + 1))
        adjT = adj[rows, :].T.astype(np.float32)  # [N, R]
        mm = np.empty((N, R), np.float32)
        nb_rows = nb * 128
        mm[:nb_rows] = adjT[:nb_rows]                      # 0/1 multiplicative
        mm[nb_rows:] = (adjT[nb_rows:] - 1.0) * 1000.0     # 0/-1000 additive
        fs = np.ascontiguousarray(f_src[rows, :].T)  # [H, R]
        d = dict(shared)
        d["maskM"] = np.ascontiguousarray(
            mm.reshape(CJ, 128, R).transpose(1, 0, 2)
        ).astype(BF16)
        d["fsrcB"] = np.broadcast_to(fs[None, :, :], (128, H, R)).astype(BF16)
        d["uB"] = np.broadcast_to(
            np.exp(fs)[None, :, :], (128, H, R)
        ).astype(BF16)
        d["u2B"] = np.broadcast_to(
            np.exp(0.2 * fs)[None, :, :], (128, H, R)
        ).astype(BF16)
        per_core.append(d)
    return per_core


def _run_layer(nc, in_maps, H, HID, tag):
    t0 = time.time()
    res = run_bass_kernel_spmd(nc, in_maps, core_ids=CORE_IDS)
    LAST_PERF[f"{tag}_wall_s"] = time.time() - t0
    LAST_PERF[f"{tag}_exec_ns"] = res.exec_time_ns

    hT = np.empty((H * HID, N), np.float32)
    for i in range(NCORES):
        a = res.results[i]["agg"]  # [H, HID+1, R]
        denom = a[:, HID : HID + 1, :]
        hT[:, R * i : R * (i + 1)] = (a[:, :HID, :] / denom).reshape(H * HID, R)
    return hT


def kernel(x, adj, W1, a1, W2, a2):
    x = np.asarray(x, np.float32)
    adj = np.asarray(adj, np.int32)
    W1 = np.asarray(W1, np.float32)
    a1 = np.asarray(a1, np.float32)
    W2 = np.asarray(W2, np.float32)
    a2 = np.asarray(a2, np.float32)

    H1, HID1, OUT = W1.shape[0], W1.shape[2], W2.shape[1]

    progA = _get_prog(H1, HID1)
    progB = _get_prog(1, OUT, NB2)

    # ---- layer 1 ----------------------------------------------------------
    W1c = np.ascontiguousarray(W1.transpose(1, 0, 2).reshape(FIN, H1 * HID1))
    wsrc1 = np.einsum("hfk,hk->fh", W1, a1[:, :HID1, 0]).astype(np.float32)
    wdst1 = np.einsum("hfk,hk->fh", W1, a1[:, HID1:, 0]).astype(np.float32)
    f_src1 = x @ wsrc1  # [N, H]
    f_dst1 = x @ wdst1
    Wh1 = x @ W1c  # [N, H1*HID1]

    in_maps = _host_inputs(f_src1, f_dst1, adj, Wh1, H1)
    hT = _run_layer(progA, in_maps, H1, HID1, "layer1")
    hcatT = _elu(hT)  # [512, N] == h_cat.T (concat=True applies elu)

    # ---- layer 2 ----------------------------------------------------------
    hcat = np.ascontiguousarray(hcatT.T)  # [N, 512]
    wsrc2 = (W2 @ a2[:OUT, 0]).astype(np.float32)[:, None]
    wdst2 = (W2 @ a2[OUT:, 0]).astype(np.float32)[:, None]
    f_src2 = hcat @ wsrc2  # [N, 1]
    f_dst2 = hcat @ wdst2
    Wh2 = hcat @ W2  # [N, OUT]
    in_maps2 = _host_inputs(f_src2, f_dst2, adj, Wh2, 1, NB2)
    outT = _run_layer(progB, in_maps2, 1, OUT, "layer2")
    # layer 2: concat=False -> no inner elu; final output = elu(out)
    return np.ascontiguousarray(_elu(outT).T)
